# revision 10
# baseline (speedup 1.0000x reference)
"""Gaussian KDE on 8 Trainium2 NeuronCores — grid (binned) KDE.

pdf[0, m, b] = sum_s exp(-||loc_m - samples_{b,s}||^2 / (2 bw^2)) / norm_b

The Gaussian kernel is separable per spatial dim, so snap the samples onto a
G x G grid with linear binning (host, O(S) prep):  W_b[gx, gy].  Then

  out[m, b] ~= sum_gx Kx[m,gx] * sum_gy Ky[m,gy] * W_b[gx,gy]

with 1-D tables Kx[m,g] = c*exp(-(lx_m - x_g)^2 / 2bw^2) (ditto Ky).  The
tables are one ACT Derivative_Erf instruction per 128-row tile (DerErf(x) =
c*exp(-x^2); the constant cancels in the final normalize).  The gy
contraction is a PE matmul (KyT stationary, W moving), and the gx contraction
is a DVE tensor_tensor_reduce.  Locations are sharded m/8 per core; the
histogram is replicated.  Norm (sum over all m) + divide happen on host
during the gather, exactly like the brute-force baseline did.

Linear binning error is O((grid step / bw)^2) ~ 2e-4 rel here, 100x inside
the 2e-2 gate and better than the fp32r brute force (2.1e-3).
"""

import os
import sys

sys.path.insert(0, "/opt/trn_rl_repo")
os.environ.setdefault("BASS_NEVER_TRACE", "1")

import numpy as np

B, S, N = 2, 4096, 2
M = 8192
N_CORES = 8
M_LOC = M // N_CORES          # 1024 locations per core
N_TILES = M_LOC // 128        # 8 partition tiles of locations
BW = 0.2
A = 1.0 / (np.sqrt(2.0) * BW)  # table arg scale: DerErf(A*(l-g)) = c*exp(-...)

GX = 128                      # x grid points (STT free dim — DVE cost ~ GX)
GY = 128                      # y grid points (matmul contraction dim)
C = GY // 128                 # gy partition chunks

_prog_cache = {}
_jit_cache = {}


def _split_excess_waits(nc):
    """This walrus build rejects >1 sync wait per instruction ("Too many sync
    wait commands"). Hoist extra waits onto NoOps inserted immediately before
    the offending instruction on the same engine queue — the engine executes
    them in order, so the wait set is identical."""
    from concourse import mybir

    for f in nc.m.functions:
        for bb in f.blocks:
            out = []
            changed = False
            for inst in bb.instructions:
                si = inst.sync_info
                waits = list(si.on_wait) if si is not None else []
                if len(waits) > 1:
                    changed = True
                    for w in waits[:-1]:
                        nop = mybir.InstNoOp(
                            name=nc.get_next_instruction_name(),
                            sync_info=mybir.SyncInfo(on_wait=[w], on_update=[]),
                            bass_nofuse=True,
                            engine=inst.engine,
                        )
                        nc.register_instruction(nop)
                        out.append(nop)
                    si.on_wait = waits[-1:]
                    inst.sync_info = si
                out.append(inst)
            if changed:
                bb.instructions = out


def build_program(reps: int = 1):
    """One NeuronCore's program. Inputs (per core):
      xg_row  [128, G]       f32: A*x_g, every partition identical
      ly_rep  [128, M_LOC]   f32: A*ly_m, every partition identical
      bias_lx [128, N_TILES] f32: -A*lx_{t*128+p}
      neg_yg  [128, C]       f32: -A*y_{c*128+p}
      wt      [128, B*C*G]   f32: col (b*C+c)*G+gx -> W_b[gx, c*128+p]
    Output: out [128, B*N_TILES] f32, col t*B+b = sum_g Kx*Ky*W (unnormalized)
    """
    key = reps
    if key in _prog_cache:
        return _prog_cache[key]

    import concourse.bass as bass
    import concourse.tile as tile
    from concourse import mybir

    f32 = mybir.dt.float32
    bf16 = mybir.dt.bfloat16
    DerErf = mybir.ActivationFunctionType.Derivative_Erf

    nc = bass.Bass()
    xg_d = nc.dram_tensor("xg_row", [128, GX], f32, kind="ExternalInput")
    ly_d = nc.dram_tensor("ly_rep", [128, M_LOC], f32, kind="ExternalInput")
    blx_d = nc.dram_tensor("bias_lx", [128, N_TILES], f32, kind="ExternalInput")
    nyg_d = nc.dram_tensor("neg_yg", [128, C], f32, kind="ExternalInput")
    wt_d = nc.dram_tensor("wt", [128, B * C * GX], bf16, kind="ExternalInput")
    out_d = nc.dram_tensor("out", [128, B * N_TILES], f32, kind="ExternalOutput")

    with tile.TileContext(nc) as tc:
        with (
            tc.tile_pool(name="consts", bufs=1) as consts,
            tc.tile_pool(name="tables", bufs=2) as tabp,
            tc.tile_pool(name="scratch", bufs=8) as scrp,
            tc.tile_pool(name="outp", bufs=2) as outp,
            tc.tile_pool(name="psum", bufs=8, space="PSUM") as psump,
        ):
            xg_t = consts.tile([128, GX], f32)
            ly_t = consts.tile([128, M_LOC], f32)
            blx_t = consts.tile([128, N_TILES], f32)
            nyg_t = consts.tile([128, C], f32)
            wt_t = consts.tile([128, B * C * GX], bf16)
            nc.sync.dma_start(xg_t[:], xg_d[:])
            nc.sync.dma_start(ly_t[:], ly_d[:])
            nc.sync.dma_start(blx_t[:], blx_d[:])
            nc.sync.dma_start(nyg_t[:], nyg_d[:])
            nc.sync.dma_start(wt_t[:], wt_d[:])

            for _ in range(reps):
                # 1-D Gaussian tables via one ACT DerErf per 128-row tile.
                kyt_t = tabp.tile([128, C, M_LOC], bf16)  # [gy_p, c, m]
                kx_t = tabp.tile([128, N_TILES, GX], f32)  # [m_p, t, gx]
                for c in range(C):
                    nc.scalar.activation(
                        out=kyt_t[:, c, :],
                        in_=ly_t[:],
                        func=DerErf,
                        bias=nyg_t[:, c : c + 1],
                        scale=1.0,
                    )
                for t in range(N_TILES):
                    nc.scalar.activation(
                        out=kx_t[:, t, :],
                        in_=xg_t[:],
                        func=DerErf,
                        bias=blx_t[:, t : t + 1],
                        scale=1.0,
                    )

                out_sb = outp.tile([128, B * N_TILES], f32)
                for t in range(N_TILES):
                    # one matmul yields U for both batches: free dim = (b, gx)
                    ps = psump.tile([128, B, GX], f32)  # U[m_p, b, gx]
                    for c in range(C):
                        nc.tensor.matmul(
                            ps[:, :, :],
                            kyt_t[:, c, t * 128 : (t + 1) * 128],
                            wt_t[:],
                            start=(c == 0),
                            stop=(c == C - 1),
                        )
                    for b in range(B):
                        prod = scrp.tile([128, GX], f32)
                        nc.vector.scalar_tensor_tensor(
                            out=prod[:],
                            in0=kx_t[:, t, :],
                            scalar=1.0,
                            in1=ps[:, b, :],
                            op0=mybir.AluOpType.mult,
                            op1=mybir.AluOpType.mult,
                            accum_out=out_sb[:, t * B + b : t * B + b + 1],
                        )
                nc.sync.dma_start(out_d[:], out_sb[:])

    _split_excess_waits(nc)
    _prog_cache[key] = nc
    return nc


def make_in_maps(samples: np.ndarray, locations: np.ndarray):
    samples = np.asarray(samples, dtype=np.float32)
    locations = np.asarray(locations, dtype=np.float32)

    flat = samples.reshape(B * S, N)
    lo = flat.min(axis=0) - 1e-4
    hi = flat.max(axis=0) + 1e-4
    step_x = (hi[0] - lo[0]) / (GX - 1)
    step_y = (hi[1] - lo[1]) / (GY - 1)
    xg = (lo[0] + step_x * np.arange(GX)).astype(np.float64)
    yg = (lo[1] + step_y * np.arange(GY)).astype(np.float64)

    # linear binning per batch -> W_b[gx, gy]
    import ml_dtypes

    wt = np.empty((128, B * C * GX), dtype=ml_dtypes.bfloat16)
    for b in range(B):
        s = samples[b].astype(np.float64)
        fx = (s[:, 0] - lo[0]) / step_x
        fy = (s[:, 1] - lo[1]) / step_y
        ix = np.clip(np.floor(fx).astype(np.int64), 0, GX - 2)
        iy = np.clip(np.floor(fy).astype(np.int64), 0, GY - 2)
        wx = np.clip(fx - ix, 0.0, 1.0)
        wy = np.clip(fy - iy, 0.0, 1.0)
        W = np.zeros((GX, GY), dtype=np.float64)
        for dx, vx in ((0, 1.0 - wx), (1, wx)):
            for dy, vy in ((0, 1.0 - wy), (1, wy)):
                np.add.at(W, (ix + dx, iy + dy), vx * vy)
        # wt[p, (b*C+c)*G+gx] = W[gx, c*128+p]
        wtb = W.T.astype(np.float32).reshape(C, 128, GX)  # [c, p, gx]
        for c in range(C):
            wt[:, (b * C + c) * GX : (b * C + c + 1) * GX] = wtb[c].astype(
                ml_dtypes.bfloat16
            )

    xg_row = np.broadcast_to((A * xg).astype(np.float32), (128, GX)).copy()
    neg_yg = (-A * yg).astype(np.float32).reshape(C, 128).T.copy()

    in_maps = []
    for core in range(N_CORES):
        lc = locations[core * M_LOC : (core + 1) * M_LOC]
        ly_rep = np.broadcast_to(
            (A * lc[:, 1]).astype(np.float32), (128, M_LOC)
        ).copy()
        bias_lx = (-A * lc[:, 0]).astype(np.float32).reshape(N_TILES, 128).T.copy()
        in_maps.append(
            {
                "xg_row": xg_row,
                "ly_rep": ly_rep,
                "bias_lx": np.ascontiguousarray(bias_lx),
                "neg_yg": neg_yg,
                "wt": wt,
            }
        )
    return in_maps


def _get_runner(nc, n_cores):
    """Build the jitted shard_map once per program. run_bass_kernel_spmd
    re-lowers the whole BIR module on every call (fresh jit closure), which
    costs ~60us/instruction of host time per call; caching the jitted
    callable leaves only device execution + fixed dispatch."""
    key = (id(nc), n_cores)
    if key in _jit_cache:
        return _jit_cache[key]

    import jax
    from jax.sharding import Mesh, PartitionSpec
    from jax.experimental.shard_map import shard_map
    from concourse import mybir
    from concourse.bass2jax import (
        _bass_exec_p,
        partition_id_tensor,
        install_neuronx_cc_hook,
    )

    install_neuronx_cc_hook()
    partition_name = nc.partition_id_tensor.name if nc.partition_id_tensor else None
    in_names, out_names, out_avals, zero_shapes = [], [], [], []
    for alloc in nc.m.functions[0].allocations:
        if not isinstance(alloc, mybir.MemoryLocationSet):
            continue
        name = alloc.memorylocations[0].name
        if alloc.kind == "ExternalInput":
            if name != partition_name:
                in_names.append(name)
        elif alloc.kind == "ExternalOutput":
            shape = tuple(alloc.tensor_shape)
            dtype = mybir.dt.np(alloc.dtype)
            out_names.append(name)
            out_avals.append(jax.core.ShapedArray(shape, dtype))
            zero_shapes.append((shape, dtype))
    n_params = len(in_names)
    all_in_names = tuple(
        in_names + out_names + ([partition_name] if partition_name else [])
    )

    def _body(*args):
        operands = list(args)
        if partition_name is not None:
            operands.append(partition_id_tensor())
        outs = _bass_exec_p.bind(
            *operands,
            out_avals=tuple(out_avals),
            in_names=all_in_names,
            out_names=tuple(out_names),
            lowering_input_output_aliases=(),
            sim_require_finite=True,
            sim_require_nnan=True,
            nc=nc,
        )
        return tuple(outs)

    donate = tuple(range(n_params, n_params + len(out_names)))
    devices = jax.devices()[:n_cores]
    mesh = Mesh(np.asarray(devices), ("core",))
    in_specs = (PartitionSpec("core"),) * (n_params + len(out_names))
    out_specs = (PartitionSpec("core"),) * len(out_names)
    sharded = jax.jit(
        shard_map(
            _body, mesh=mesh, in_specs=in_specs, out_specs=out_specs,
            check_rep=False,
        ),
        donate_argnums=donate,
        keep_unused=True,
    )
    r = (sharded, in_names, out_names, out_avals, zero_shapes)
    _jit_cache[key] = r
    return r


_input_cache = {}


def run_on_cores(in_maps, reps: int = 1):
    import jax
    from jax.sharding import NamedSharding, PartitionSpec

    nc = build_program(reps)
    sharded, in_names, out_names, out_avals, zero_shapes = _get_runner(nc, N_CORES)
    # Upload inputs once per (program, in_maps) pair — repeated timing calls
    # would otherwise re-ship ~9MB over the axon tunnel every call.
    ikey = (id(nc), tuple(id(m[name]) for m in in_maps for name in in_names))
    concat_in = _input_cache.get(ikey)
    if concat_in is None:
        concat_in = [
            np.concatenate([np.asarray(m[name]) for m in in_maps], axis=0)
            for name in in_names
        ]
        try:
            from jax.sharding import Mesh

            devices = jax.devices()[:N_CORES]
            mesh = Mesh(np.asarray(devices), ("core",))
            sh = NamedSharding(mesh, PartitionSpec("core"))
            concat_in = [jax.device_put(a, sh) for a in concat_in]
        except Exception:
            pass
        _input_cache[ikey] = concat_in
    concat_zeros = [
        np.zeros((N_CORES * shape[0], *shape[1:]), dtype)
        for shape, dtype in zero_shapes
    ]
    out_arrs = sharded(*concat_in, *concat_zeros)
    return [
        {
            name: np.asarray(out_arrs[i]).reshape(N_CORES, *out_avals[i].shape)[c]
            for i, name in enumerate(out_names)
        }
        for c in range(N_CORES)
    ]


def kernel(samples: np.ndarray, locations: np.ndarray) -> np.ndarray:
    in_maps = make_in_maps(samples, locations)
    res = run_on_cores(in_maps, reps=1)
    # out core c: [128, B*N_TILES], col t*B+b -> m = c*M_LOC + t*128 + p
    out_full = np.empty((M, B), dtype=np.float32)
    for c in range(N_CORES):
        o = res[c]["out"]  # [128, 16]
        o = o.reshape(128, N_TILES, B).transpose(1, 0, 2)  # [t, p, b]
        out_full[c * M_LOC : (c + 1) * M_LOC] = o.reshape(M_LOC, B)
    norm = out_full.sum(axis=0)
    pdf = (out_full / norm.reshape(1, B)).reshape(1, M, B)
    return pdf.astype(np.float32)


# revision 18
# speedup vs baseline: 1.3527x; 1.3527x over previous
"""Gaussian KDE on 8 Trainium2 NeuronCores — grid (binned) KDE.

pdf[0, m, b] = sum_s exp(-||loc_m - samples_{b,s}||^2 / (2 bw^2)) / norm_b

The Gaussian kernel is separable per spatial dim, so snap the samples onto a
GX x GY grid with linear binning (host, O(S) prep):  W_b[gx, gy].  Then

  out[m, b] ~= sum_gx Kx[m,gx] * sum_gy Ky[m,gy] * W_b[gx,gy]

with 1-D tables Kx[m,g] = c*exp(-(lx_m - x_g)^2 / 2bw^2) (ditto Ky).  The
tables are one ACT Derivative_Erf instruction per 128-row tile (DerErf(x) =
c*exp(-x^2); the constant cancels in the final normalize).  The gy
contraction is a PE matmul (KyT stationary, W moving, bf16), and the gx
contraction is a DVE scalar_tensor_tensor with accum_out (fused
multiply-reduce).  Locations are sharded m/8 per core; the histogram is
replicated.  Norm (sum over all m) + divide happen on host during the
gather, exactly like the brute-force baseline did.

Linear binning error is O((grid step / bw)^2); with 128x128 + bf16 matmul
operands the end-to-end rel error is 9.2e-4, 20x inside the 2e-2 gate and
2.3x better than the fp32r brute force (2.1e-3).  Per-rep device time ~4us
vs 66us for the brute-force kernel (ACT-exp bound).

Steady-state the kernel is DVE-bound: 16 STT ops x (128 elems + the 2x120
cycle PSUM access init) ~= 4.1us.  Blocked cheaper alternatives on this
walrus build: tensor_tensor_reduce fails codegen ("ISA wrong length"),
GPSIMD rejects scalar_tensor_tensor and cannot access PSUM, DVE ops accept
at most one PSUM input, DMA cannot read PSUM, and TensorScalarPtr has no
2x perf mode — so U cannot be staged to SBUF without loading ACT (the
next-slowest engine) past the DVE time it would save.
"""

import os
import sys

sys.path.insert(0, "/opt/trn_rl_repo")
os.environ.setdefault("BASS_NEVER_TRACE", "1")

import numpy as np

B, S, N = 2, 4096, 2
M = 8192
N_CORES = 8
M_LOC = M // N_CORES          # 1024 locations per core
N_TILES = M_LOC // 128        # 8 partition tiles of locations
BW = 0.2
A = 1.0 / (np.sqrt(2.0) * BW)  # table arg scale: DerErf(A*(l-g)) = c*exp(-...)

GX = 128                      # x grid points (STT free dim — DVE cost ~ GX)
GY = 128                      # y grid points (matmul contraction dim)
C = GY // 128                 # gy partition chunks

_prog_cache = {}
_jit_cache = {}


def _split_excess_waits(nc):
    """This walrus build rejects >1 sync wait per instruction ("Too many sync
    wait commands"). Hoist extra waits onto NoOps inserted immediately before
    the offending instruction on the same engine queue — the engine executes
    them in order, so the wait set is identical."""
    from concourse import mybir

    for f in nc.m.functions:
        for bb in f.blocks:
            out = []
            changed = False
            for inst in bb.instructions:
                si = inst.sync_info
                waits = list(si.on_wait) if si is not None else []
                if len(waits) > 1:
                    changed = True
                    for w in waits[:-1]:
                        nop = mybir.InstNoOp(
                            name=nc.get_next_instruction_name(),
                            sync_info=mybir.SyncInfo(on_wait=[w], on_update=[]),
                            bass_nofuse=True,
                            engine=inst.engine,
                        )
                        nc.register_instruction(nop)
                        out.append(nop)
                    si.on_wait = waits[-1:]
                    inst.sync_info = si
                out.append(inst)
            if changed:
                bb.instructions = out


def build_program(reps: int = 1):
    """One NeuronCore's program. Inputs (per core):
      xg_row  [128, GX]      f32: A*x_g, every partition identical
      ly_rep  [128, M_LOC]   f32: A*ly_m, every partition identical
      bias_lx [128, N_TILES] f32: -A*lx_{t*128+p}
      neg_yg  [128, C]       f32: -A*y_{c*128+p}
      wt      [128, B*C*GX]  bf16: col (b*C+c)*GX+gx -> W_b[gx, c*128+p]
    Output: out [128, B*N_TILES] f32, col t*B+b = sum_g Kx*Ky*W (unnormalized)
    """
    key = (reps, GX, GY)
    if key in _prog_cache:
        return _prog_cache[key]

    import concourse.bass as bass
    import concourse.tile as tile
    from concourse import mybir

    f32 = mybir.dt.float32
    bf16 = mybir.dt.bfloat16
    DerErf = mybir.ActivationFunctionType.Derivative_Erf

    nc = bass.Bass()
    xg_d = nc.dram_tensor("xg_row", [128, GX], f32, kind="ExternalInput")
    ly_d = nc.dram_tensor("ly_rep", [128, M_LOC], f32, kind="ExternalInput")
    blx_d = nc.dram_tensor("bias_lx", [128, N_TILES], f32, kind="ExternalInput")
    nyg_d = nc.dram_tensor("neg_yg", [128, C], f32, kind="ExternalInput")
    wt_d = nc.dram_tensor("wt", [128, B * C * GX], bf16, kind="ExternalInput")
    out_d = nc.dram_tensor("out", [128, B * N_TILES], f32, kind="ExternalOutput")

    with tile.TileContext(nc) as tc:
        with (
            tc.tile_pool(name="consts", bufs=1) as consts,
            tc.tile_pool(name="tables", bufs=2) as tabp,
            tc.tile_pool(name="scratch", bufs=8) as scrp,
            tc.tile_pool(name="outp", bufs=2) as outp,
            tc.tile_pool(name="psum", bufs=8, space="PSUM") as psump,
        ):
            xg_t = consts.tile([128, GX], f32)
            ly_t = consts.tile([128, M_LOC], f32)
            blx_t = consts.tile([128, N_TILES], f32)
            nyg_t = consts.tile([128, C], f32)
            wt_t = consts.tile([128, B * C * GX], bf16)
            nc.sync.dma_start(xg_t[:], xg_d[:])
            nc.sync.dma_start(ly_t[:], ly_d[:])
            nc.sync.dma_start(blx_t[:], blx_d[:])
            nc.sync.dma_start(nyg_t[:], nyg_d[:])
            nc.sync.dma_start(wt_t[:], wt_d[:])

            for _ in range(reps):
                # 1-D Gaussian tables via one ACT DerErf per 128-row tile.
                kyt_t = tabp.tile([128, C, M_LOC], bf16)  # [gy_p, c, m]
                kx_t = tabp.tile([128, N_TILES, GX], f32)  # [m_p, t, gx]
                for c in range(C):
                    nc.scalar.activation(
                        out=kyt_t[:, c, :],
                        in_=ly_t[:],
                        func=DerErf,
                        bias=nyg_t[:, c : c + 1],
                        scale=1.0,
                    )
                for t in range(N_TILES):
                    nc.scalar.activation(
                        out=kx_t[:, t, :],
                        in_=xg_t[:],
                        func=DerErf,
                        bias=blx_t[:, t : t + 1],
                        scale=1.0,
                    )

                out_sb = outp.tile([128, B * N_TILES], f32)
                for t in range(N_TILES):
                    # one matmul yields U for both batches: free dim = (b, gx)
                    ps = psump.tile([128, B, GX], f32)  # U[m_p, b, gx]
                    for c in range(C):
                        nc.tensor.matmul(
                            ps[:, :, :],
                            kyt_t[:, c, t * 128 : (t + 1) * 128],
                            wt_t[:],
                            start=(c == 0),
                            stop=(c == C - 1),
                        )
                    for b in range(B):
                        prod = scrp.tile([128, GX], f32)
                        nc.vector.scalar_tensor_tensor(
                            out=prod[:],
                            in0=kx_t[:, t, :],
                            scalar=1.0,
                            in1=ps[:, b, :],
                            op0=mybir.AluOpType.mult,
                            op1=mybir.AluOpType.mult,
                            accum_out=out_sb[:, t * B + b : t * B + b + 1],
                        )
                nc.sync.dma_start(out_d[:], out_sb[:])

    _split_excess_waits(nc)
    _prog_cache[key] = nc
    return nc


def make_in_maps(samples: np.ndarray, locations: np.ndarray):
    samples = np.asarray(samples, dtype=np.float32)
    locations = np.asarray(locations, dtype=np.float32)

    flat = samples.reshape(B * S, N)
    lo = flat.min(axis=0) - 1e-4
    hi = flat.max(axis=0) + 1e-4
    step_x = (hi[0] - lo[0]) / (GX - 1)
    step_y = (hi[1] - lo[1]) / (GY - 1)
    xg = (lo[0] + step_x * np.arange(GX)).astype(np.float64)
    yg = (lo[1] + step_y * np.arange(GY)).astype(np.float64)

    # linear binning per batch -> W_b[gx, gy]
    import ml_dtypes

    wt = np.empty((128, B * C * GX), dtype=ml_dtypes.bfloat16)
    for b in range(B):
        s = samples[b].astype(np.float64)
        fx = (s[:, 0] - lo[0]) / step_x
        fy = (s[:, 1] - lo[1]) / step_y
        ix = np.clip(np.floor(fx).astype(np.int64), 0, GX - 2)
        iy = np.clip(np.floor(fy).astype(np.int64), 0, GY - 2)
        wx = np.clip(fx - ix, 0.0, 1.0)
        wy = np.clip(fy - iy, 0.0, 1.0)
        W = np.zeros((GX, GY), dtype=np.float64)
        for dx, vx in ((0, 1.0 - wx), (1, wx)):
            for dy, vy in ((0, 1.0 - wy), (1, wy)):
                np.add.at(W, (ix + dx, iy + dy), vx * vy)
        # wt[p, (b*C+c)*GX+gx] = W[gx, c*128+p]
        wtb = W.T.astype(np.float32).reshape(C, 128, GX)  # [c, p, gx]
        for c in range(C):
            wt[:, (b * C + c) * GX : (b * C + c + 1) * GX] = wtb[c].astype(
                ml_dtypes.bfloat16
            )

    xg_row = np.broadcast_to((A * xg).astype(np.float32), (128, GX)).copy()
    neg_yg = (-A * yg).astype(np.float32).reshape(C, 128).T.copy()

    in_maps = []
    for core in range(N_CORES):
        lc = locations[core * M_LOC : (core + 1) * M_LOC]
        ly_rep = np.broadcast_to(
            (A * lc[:, 1]).astype(np.float32), (128, M_LOC)
        ).copy()
        bias_lx = (-A * lc[:, 0]).astype(np.float32).reshape(N_TILES, 128).T.copy()
        in_maps.append(
            {
                "xg_row": xg_row,
                "ly_rep": ly_rep,
                "bias_lx": np.ascontiguousarray(bias_lx),
                "neg_yg": neg_yg,
                "wt": wt,
            }
        )
    return in_maps


def _get_runner(nc, n_cores):
    """Build the jitted shard_map once per program. run_bass_kernel_spmd
    re-lowers the whole BIR module on every call (fresh jit closure), which
    costs ~60us/instruction of host time per call; caching the jitted
    callable leaves only device execution + fixed dispatch."""
    key = (id(nc), n_cores)
    if key in _jit_cache:
        return _jit_cache[key]

    import jax
    from jax.sharding import Mesh, PartitionSpec
    from jax.experimental.shard_map import shard_map
    from concourse import mybir
    from concourse.bass2jax import (
        _bass_exec_p,
        partition_id_tensor,
        install_neuronx_cc_hook,
    )

    install_neuronx_cc_hook()
    partition_name = nc.partition_id_tensor.name if nc.partition_id_tensor else None
    in_names, out_names, out_avals, zero_shapes = [], [], [], []
    for alloc in nc.m.functions[0].allocations:
        if not isinstance(alloc, mybir.MemoryLocationSet):
            continue
        name = alloc.memorylocations[0].name
        if alloc.kind == "ExternalInput":
            if name != partition_name:
                in_names.append(name)
        elif alloc.kind == "ExternalOutput":
            shape = tuple(alloc.tensor_shape)
            dtype = mybir.dt.np(alloc.dtype)
            out_names.append(name)
            out_avals.append(jax.core.ShapedArray(shape, dtype))
            zero_shapes.append((shape, dtype))
    n_params = len(in_names)
    all_in_names = tuple(
        in_names + out_names + ([partition_name] if partition_name else [])
    )

    def _body(*args):
        operands = list(args)
        if partition_name is not None:
            operands.append(partition_id_tensor())
        outs = _bass_exec_p.bind(
            *operands,
            out_avals=tuple(out_avals),
            in_names=all_in_names,
            out_names=tuple(out_names),
            lowering_input_output_aliases=(),
            sim_require_finite=True,
            sim_require_nnan=True,
            nc=nc,
        )
        return tuple(outs)

    donate = tuple(range(n_params, n_params + len(out_names)))
    devices = jax.devices()[:n_cores]
    mesh = Mesh(np.asarray(devices), ("core",))
    in_specs = (PartitionSpec("core"),) * (n_params + len(out_names))
    out_specs = (PartitionSpec("core"),) * len(out_names)
    sharded = jax.jit(
        shard_map(
            _body, mesh=mesh, in_specs=in_specs, out_specs=out_specs,
            check_rep=False,
        ),
        donate_argnums=donate,
        keep_unused=True,
    )
    r = (sharded, in_names, out_names, out_avals, zero_shapes)
    _jit_cache[key] = r
    return r


_input_cache = {}


def run_on_cores(in_maps, reps: int = 1):
    import jax
    from jax.sharding import Mesh, NamedSharding, PartitionSpec

    nc = build_program(reps)
    sharded, in_names, out_names, out_avals, zero_shapes = _get_runner(nc, N_CORES)
    # Upload inputs once per (program, in_maps) pair — repeated timing calls
    # would otherwise re-ship the inputs over the axon tunnel every call.
    ikey = (id(nc), tuple(id(m[name]) for m in in_maps for name in in_names))
    concat_in = _input_cache.get(ikey)
    if concat_in is None:
        concat_in = [
            np.concatenate([np.asarray(m[name]) for m in in_maps], axis=0)
            for name in in_names
        ]
        try:
            devices = jax.devices()[:N_CORES]
            mesh = Mesh(np.asarray(devices), ("core",))
            sh = NamedSharding(mesh, PartitionSpec("core"))
            concat_in = [jax.device_put(a, sh) for a in concat_in]
        except Exception:
            pass
        _input_cache[ikey] = concat_in
    concat_zeros = [
        np.zeros((N_CORES * shape[0], *shape[1:]), dtype)
        for shape, dtype in zero_shapes
    ]
    out_arrs = sharded(*concat_in, *concat_zeros)
    return [
        {
            name: np.asarray(out_arrs[i]).reshape(N_CORES, *out_avals[i].shape)[c]
            for i, name in enumerate(out_names)
        }
        for c in range(N_CORES)
    ]


def kernel(samples: np.ndarray, locations: np.ndarray) -> np.ndarray:
    in_maps = make_in_maps(samples, locations)
    res = run_on_cores(in_maps, reps=1)
    # out core c: [128, B*N_TILES], col t*B+b -> m = c*M_LOC + t*128 + p
    out_full = np.empty((M, B), dtype=np.float32)
    for c in range(N_CORES):
        o = res[c]["out"]  # [128, 16]
        o = o.reshape(128, N_TILES, B).transpose(1, 0, 2)  # [t, p, b]
        out_full[c * M_LOC : (c + 1) * M_LOC] = o.reshape(M_LOC, B)
    norm = out_full.sum(axis=0)
    pdf = (out_full / norm.reshape(1, B)).reshape(1, M, B)
    return pdf.astype(np.float32)


# revision 21
# speedup vs baseline: 1.3985x; 1.0338x over previous
"""Gaussian KDE on 8 Trainium2 NeuronCores — grid (binned) KDE.

pdf[0, m, b] = sum_s exp(-||loc_m - samples_{b,s}||^2 / (2 bw^2)) / norm_b

The Gaussian kernel is separable per spatial dim, so snap the samples onto a
GX x GY grid with linear binning (host, O(S) prep):  W_b[gx, gy].  Then

  out[m, b] ~= sum_gx Kx[m,gx] * sum_gy Ky[m,gy] * W_b[gx,gy]

with 1-D tables Kx[m,g] = c*exp(-(lx_m - x_g)^2 / 2bw^2) (ditto Ky).  The
tables are one ACT Derivative_Erf instruction per 128-row tile (DerErf(x) =
c*exp(-x^2); the constant cancels in the final normalize).  The gy
contraction is a PE matmul (KyT stationary, W moving, bf16), and the gx
contraction is a DVE scalar_tensor_tensor with accum_out (fused
multiply-reduce).  Locations are sharded m/8 per core; the histogram is
replicated.  Norm (sum over all m) + divide happen on host during the
gather, exactly like the brute-force baseline did.

Linear binning error is O((grid step / bw)^2); with 96x128 + bf16 matmul
operands the end-to-end rel error is 1.2e-3, 17x inside the 2e-2 gate and
still better than the fp32r brute force (2.1e-3) on both the l2 and absmax
metrics.  Per-rep device time ~3.9us vs 66us for the brute-force kernel.

Steady-state the kernel is DVE-bound: 16 STT ops x (GX elems + the 2x120
cycle PSUM access init).  GX=96 measured ~10% faster than GX=128 in a
24-pair alternating A/B at reps=2049.  Blocked cheaper alternatives on
this walrus build: tensor_tensor_reduce fails codegen ("ISA wrong
length"), GPSIMD rejects scalar_tensor_tensor and cannot access PSUM, DVE
ops accept at most one PSUM input, DMA cannot read PSUM, and
TensorScalarPtr has no 2x perf mode — so U cannot be staged to SBUF
without loading ACT past the DVE time it would save.  The Kx args are
formed by the otherwise-idle Pool engine (tensor_sub) so ACT runs only 3
table ops (~2.1us busy).
"""

import os
import sys

sys.path.insert(0, "/opt/trn_rl_repo")
os.environ.setdefault("BASS_NEVER_TRACE", "1")

import numpy as np

B, S, N = 2, 4096, 2
M = 8192
N_CORES = 8
M_LOC = M // N_CORES          # 1024 locations per core
N_TILES = M_LOC // 128        # 8 partition tiles of locations
BW = 0.2
A = 1.0 / (np.sqrt(2.0) * BW)  # table arg scale: DerErf(A*(l-g)) = c*exp(-...)

GX = 96                       # x grid points (STT free dim — DVE cost ~ GX)
GY = 128                      # y grid points (matmul contraction dim)
C = GY // 128                 # gy partition chunks

_prog_cache = {}
_jit_cache = {}


def _split_excess_waits(nc):
    """This walrus build rejects >1 sync wait per instruction ("Too many sync
    wait commands"). Hoist extra waits onto NoOps inserted immediately before
    the offending instruction on the same engine queue — the engine executes
    them in order, so the wait set is identical."""
    from concourse import mybir

    for f in nc.m.functions:
        for bb in f.blocks:
            out = []
            changed = False
            for inst in bb.instructions:
                si = inst.sync_info
                waits = list(si.on_wait) if si is not None else []
                if len(waits) > 1:
                    changed = True
                    for w in waits[:-1]:
                        nop = mybir.InstNoOp(
                            name=nc.get_next_instruction_name(),
                            sync_info=mybir.SyncInfo(on_wait=[w], on_update=[]),
                            bass_nofuse=True,
                            engine=inst.engine,
                        )
                        nc.register_instruction(nop)
                        out.append(nop)
                    si.on_wait = waits[-1:]
                    inst.sync_info = si
                out.append(inst)
            if changed:
                bb.instructions = out


def build_program(reps: int = 1):
    """One NeuronCore's program. Inputs (per core):
      xg8     [128, NT*GX]   f32: A*x_g tiled NT times (partition-identical)
      lxr     [128, NT*GX]   f32: A*lx_{t*128+p} repeated GX times per t
      ly_rep  [128, M_LOC]   f32: A*ly_m, every partition identical
      neg_yg  [128, C]       f32: -A*y_{c*128+p}
      wt      [128, B*C*GX]  bf16: col (b*C+c)*GX+gx -> W_b[gx, c*128+p]
    Output: out [128, B*N_TILES] f32, col t*B+b = sum_g Kx*Ky*W (unnormalized)
    """
    key = (reps, GX, GY)
    if key in _prog_cache:
        return _prog_cache[key]

    import concourse.bass as bass
    import concourse.tile as tile
    from concourse import mybir

    f32 = mybir.dt.float32
    bf16 = mybir.dt.bfloat16
    DerErf = mybir.ActivationFunctionType.Derivative_Erf

    nc = bass.Bass()
    xg_d = nc.dram_tensor("xg8", [128, N_TILES * GX], f32, kind="ExternalInput")
    lxr_d = nc.dram_tensor("lxr", [128, N_TILES * GX], f32, kind="ExternalInput")
    ly_d = nc.dram_tensor("ly_rep", [128, M_LOC], f32, kind="ExternalInput")
    nyg_d = nc.dram_tensor("neg_yg", [128, C], f32, kind="ExternalInput")
    wt_d = nc.dram_tensor("wt", [128, B * C * GX], bf16, kind="ExternalInput")
    out_d = nc.dram_tensor("out", [128, B * N_TILES], f32, kind="ExternalOutput")

    with tile.TileContext(nc) as tc:
        with (
            tc.tile_pool(name="consts", bufs=1) as consts,
            tc.tile_pool(name="tables", bufs=2) as tabp,
            tc.tile_pool(name="scratch", bufs=8) as scrp,
            tc.tile_pool(name="outp", bufs=2) as outp,
            tc.tile_pool(name="psum", bufs=8, space="PSUM") as psump,
        ):
            xg_t = consts.tile([128, N_TILES * GX], f32)
            lxr_t = consts.tile([128, N_TILES * GX], f32)
            ly_t = consts.tile([128, M_LOC], f32)
            nyg_t = consts.tile([128, C], f32)
            wt_t = consts.tile([128, B * C * GX], bf16)
            nc.sync.dma_start(xg_t[:], xg_d[:])
            nc.sync.dma_start(lxr_t[:], lxr_d[:])
            nc.sync.dma_start(ly_t[:], ly_d[:])
            nc.sync.dma_start(nyg_t[:], nyg_d[:])
            nc.sync.dma_start(wt_t[:], wt_d[:])

            for _ in range(reps):
                # 1-D Gaussian tables.  Ky: one ACT DerErf per gy chunk
                # (bias = per-partition -A*y_g).  Kx: the idle Pool engine
                # forms all 8 tiles' args (A*x_g - A*lx_m) in one tensor_sub,
                # then ONE big ACT DerErf converts them — 2 ACT ops total
                # instead of 9, cutting ACT busy from ~3.4us to ~2.1us.
                kyt_t = tabp.tile([128, C, M_LOC], bf16)  # [gy_p, c, m]
                args_t = tabp.tile([128, N_TILES, GX], f32)
                kx_t = tabp.tile([128, N_TILES, GX], f32)  # [m_p, t, gx]
                for c in range(C):
                    nc.scalar.activation(
                        out=kyt_t[:, c, :],
                        in_=ly_t[:],
                        func=DerErf,
                        bias=nyg_t[:, c : c + 1],
                        scale=1.0,
                    )
                nc.gpsimd.tensor_sub(args_t[:, :, :], xg_t[:], lxr_t[:])
                nc.scalar.activation(
                    out=kx_t[:, :, :],
                    in_=args_t[:, :, :],
                    func=DerErf,
                    scale=1.0,
                )

                out_sb = outp.tile([128, B * N_TILES], f32)
                for t in range(N_TILES):
                    # one matmul yields U for both batches: free dim = (b, gx)
                    ps = psump.tile([128, B, GX], f32)  # U[m_p, b, gx]
                    for c in range(C):
                        nc.tensor.matmul(
                            ps[:, :, :],
                            kyt_t[:, c, t * 128 : (t + 1) * 128],
                            wt_t[:],
                            start=(c == 0),
                            stop=(c == C - 1),
                        )
                    for b in range(B):
                        prod = scrp.tile([128, GX], f32)
                        nc.vector.scalar_tensor_tensor(
                            out=prod[:],
                            in0=kx_t[:, t, :],
                            scalar=1.0,
                            in1=ps[:, b, :],
                            op0=mybir.AluOpType.mult,
                            op1=mybir.AluOpType.mult,
                            accum_out=out_sb[:, t * B + b : t * B + b + 1],
                        )
                nc.sync.dma_start(out_d[:], out_sb[:])

    _split_excess_waits(nc)
    _prog_cache[key] = nc
    return nc


def make_in_maps(samples: np.ndarray, locations: np.ndarray):
    samples = np.asarray(samples, dtype=np.float32)
    locations = np.asarray(locations, dtype=np.float32)

    flat = samples.reshape(B * S, N)
    lo = flat.min(axis=0) - 1e-4
    hi = flat.max(axis=0) + 1e-4
    step_x = (hi[0] - lo[0]) / (GX - 1)
    step_y = (hi[1] - lo[1]) / (GY - 1)
    xg = (lo[0] + step_x * np.arange(GX)).astype(np.float64)
    yg = (lo[1] + step_y * np.arange(GY)).astype(np.float64)

    # linear binning per batch -> W_b[gx, gy]
    import ml_dtypes

    wt = np.empty((128, B * C * GX), dtype=ml_dtypes.bfloat16)
    for b in range(B):
        s = samples[b].astype(np.float64)
        fx = (s[:, 0] - lo[0]) / step_x
        fy = (s[:, 1] - lo[1]) / step_y
        ix = np.clip(np.floor(fx).astype(np.int64), 0, GX - 2)
        iy = np.clip(np.floor(fy).astype(np.int64), 0, GY - 2)
        wx = np.clip(fx - ix, 0.0, 1.0)
        wy = np.clip(fy - iy, 0.0, 1.0)
        W = np.zeros((GX, GY), dtype=np.float64)
        for dx, vx in ((0, 1.0 - wx), (1, wx)):
            for dy, vy in ((0, 1.0 - wy), (1, wy)):
                np.add.at(W, (ix + dx, iy + dy), vx * vy)
        # wt[p, (b*C+c)*GX+gx] = W[gx, c*128+p]
        wtb = W.T.astype(np.float32).reshape(C, 128, GX)  # [c, p, gx]
        for c in range(C):
            wt[:, (b * C + c) * GX : (b * C + c + 1) * GX] = wtb[c].astype(
                ml_dtypes.bfloat16
            )

    xg8 = np.broadcast_to(
        np.tile((A * xg).astype(np.float32), N_TILES), (128, N_TILES * GX)
    ).copy()
    neg_yg = (-A * yg).astype(np.float32).reshape(C, 128).T.copy()

    in_maps = []
    for core in range(N_CORES):
        lc = locations[core * M_LOC : (core + 1) * M_LOC]
        ly_rep = np.broadcast_to(
            (A * lc[:, 1]).astype(np.float32), (128, M_LOC)
        ).copy()
        lxt = (A * lc[:, 0]).astype(np.float32).reshape(N_TILES, 128).T  # [p, t]
        lxr = np.repeat(lxt[:, :, None], GX, axis=2).reshape(128, N_TILES * GX)
        in_maps.append(
            {
                "xg8": xg8,
                "lxr": np.ascontiguousarray(lxr),
                "ly_rep": ly_rep,
                "neg_yg": neg_yg,
                "wt": wt,
            }
        )
    return in_maps


def _get_runner(nc, n_cores):
    """Build the jitted shard_map once per program. run_bass_kernel_spmd
    re-lowers the whole BIR module on every call (fresh jit closure), which
    costs ~60us/instruction of host time per call; caching the jitted
    callable leaves only device execution + fixed dispatch."""
    key = (id(nc), n_cores)
    if key in _jit_cache:
        return _jit_cache[key]

    import jax
    from jax.sharding import Mesh, PartitionSpec
    from jax.experimental.shard_map import shard_map
    from concourse import mybir
    from concourse.bass2jax import (
        _bass_exec_p,
        partition_id_tensor,
        install_neuronx_cc_hook,
    )

    install_neuronx_cc_hook()
    partition_name = nc.partition_id_tensor.name if nc.partition_id_tensor else None
    in_names, out_names, out_avals, zero_shapes = [], [], [], []
    for alloc in nc.m.functions[0].allocations:
        if not isinstance(alloc, mybir.MemoryLocationSet):
            continue
        name = alloc.memorylocations[0].name
        if alloc.kind == "ExternalInput":
            if name != partition_name:
                in_names.append(name)
        elif alloc.kind == "ExternalOutput":
            shape = tuple(alloc.tensor_shape)
            dtype = mybir.dt.np(alloc.dtype)
            out_names.append(name)
            out_avals.append(jax.core.ShapedArray(shape, dtype))
            zero_shapes.append((shape, dtype))
    n_params = len(in_names)
    all_in_names = tuple(
        in_names + out_names + ([partition_name] if partition_name else [])
    )

    def _body(*args):
        operands = list(args)
        if partition_name is not None:
            operands.append(partition_id_tensor())
        outs = _bass_exec_p.bind(
            *operands,
            out_avals=tuple(out_avals),
            in_names=all_in_names,
            out_names=tuple(out_names),
            lowering_input_output_aliases=(),
            sim_require_finite=True,
            sim_require_nnan=True,
            nc=nc,
        )
        return tuple(outs)

    donate = tuple(range(n_params, n_params + len(out_names)))
    devices = jax.devices()[:n_cores]
    mesh = Mesh(np.asarray(devices), ("core",))
    in_specs = (PartitionSpec("core"),) * (n_params + len(out_names))
    out_specs = (PartitionSpec("core"),) * len(out_names)
    sharded = jax.jit(
        shard_map(
            _body, mesh=mesh, in_specs=in_specs, out_specs=out_specs,
            check_rep=False,
        ),
        donate_argnums=donate,
        keep_unused=True,
    )
    r = (sharded, in_names, out_names, out_avals, zero_shapes)
    _jit_cache[key] = r
    return r


_input_cache = {}


def run_on_cores(in_maps, reps: int = 1):
    import jax
    from jax.sharding import Mesh, NamedSharding, PartitionSpec

    nc = build_program(reps)
    sharded, in_names, out_names, out_avals, zero_shapes = _get_runner(nc, N_CORES)
    # Upload inputs once per (program, in_maps) pair — repeated timing calls
    # would otherwise re-ship the inputs over the axon tunnel every call.
    ikey = (id(nc), tuple(id(m[name]) for m in in_maps for name in in_names))
    concat_in = _input_cache.get(ikey)
    if concat_in is None:
        concat_in = [
            np.concatenate([np.asarray(m[name]) for m in in_maps], axis=0)
            for name in in_names
        ]
        try:
            devices = jax.devices()[:N_CORES]
            mesh = Mesh(np.asarray(devices), ("core",))
            sh = NamedSharding(mesh, PartitionSpec("core"))
            concat_in = [jax.device_put(a, sh) for a in concat_in]
        except Exception:
            pass
        _input_cache[ikey] = concat_in
    concat_zeros = [
        np.zeros((N_CORES * shape[0], *shape[1:]), dtype)
        for shape, dtype in zero_shapes
    ]
    out_arrs = sharded(*concat_in, *concat_zeros)
    return [
        {
            name: np.asarray(out_arrs[i]).reshape(N_CORES, *out_avals[i].shape)[c]
            for i, name in enumerate(out_names)
        }
        for c in range(N_CORES)
    ]


def kernel(samples: np.ndarray, locations: np.ndarray) -> np.ndarray:
    in_maps = make_in_maps(samples, locations)
    res = run_on_cores(in_maps, reps=1)
    # out core c: [128, B*N_TILES], col t*B+b -> m = c*M_LOC + t*128 + p
    out_full = np.empty((M, B), dtype=np.float32)
    for c in range(N_CORES):
        o = res[c]["out"]  # [128, 16]
        o = o.reshape(128, N_TILES, B).transpose(1, 0, 2)  # [t, p, b]
        out_full[c * M_LOC : (c + 1) * M_LOC] = o.reshape(M_LOC, B)
    norm = out_full.sum(axis=0)
    pdf = (out_full / norm.reshape(1, B)).reshape(1, M, B)
    return pdf.astype(np.float32)


# revision 23
# speedup vs baseline: 1.8798x; 1.3441x over previous
"""Gaussian KDE on 8 Trainium2 NeuronCores — grid (binned) KDE.

pdf[0, m, b] = sum_s exp(-||loc_m - samples_{b,s}||^2 / (2 bw^2)) / norm_b

The Gaussian kernel is separable per spatial dim, so snap the samples onto a
GX x GY grid with linear binning (host, O(S) prep):  W_b[gx, gy].  Then

  out[m, b] ~= sum_gx Kx[m,gx] * sum_gy Ky[m,gy] * W_b[gx,gy]

with 1-D tables Kx[m,g] = c*exp(-(lx_m - x_g)^2 / 2bw^2) (ditto Ky).  The
tables are one ACT Derivative_Erf instruction per 128-row tile (DerErf(x) =
c*exp(-x^2); the constant cancels in the final normalize).  The gy
contraction is a PE matmul (KyT stationary, W moving, bf16), and the gx
contraction is a DVE scalar_tensor_tensor with accum_out (fused
multiply-reduce).  Locations are sharded m/8 per core; the histogram is
replicated.  Norm (sum over all m) + divide happen on host during the
gather, exactly like the brute-force baseline did.

Linear binning error is O((grid step / bw)^2); with 96x128 + bf16 matmul
operands the end-to-end rel error is 1.2e-3, 17x inside the 2e-2 gate and
still better than the fp32r brute force (2.1e-3) on both the l2 and absmax
metrics.  Per-rep device time ~3.9us vs 66us for the brute-force kernel.

Steady-state the kernel is DVE-bound: 16 STT ops x (GX elems + the PSUM
access init).  GX=96 measured ~10% faster than GX=128 in a 24-pair
alternating A/B at reps=2049.  ACT's ~1.5us of slack is spent staging 3 of
the 8 tiles' U into SBUF (exact f32 Copy — same act-table set as DerErf,
no reload), halving those STTs' access init; measured 3232->2770 ns in a
40-pair A/B, matching the cost model's predicted balance point.  Blocked cheaper alternatives on
this walrus build: tensor_tensor_reduce fails codegen ("ISA wrong
length"), GPSIMD rejects scalar_tensor_tensor and cannot access PSUM, DVE
ops accept at most one PSUM input, DMA cannot read PSUM, and
TensorScalarPtr has no 2x perf mode — so U cannot be staged to SBUF
without loading ACT past the DVE time it would save.  The Kx args are
formed by the otherwise-idle Pool engine (tensor_sub) so ACT runs only 3
table ops (~2.1us busy).
"""

import os
import sys

sys.path.insert(0, "/opt/trn_rl_repo")
os.environ.setdefault("BASS_NEVER_TRACE", "1")

import numpy as np

B, S, N = 2, 4096, 2
M = 8192
N_CORES = 8
M_LOC = M // N_CORES          # 1024 locations per core
N_TILES = M_LOC // 128        # 8 partition tiles of locations
BW = 0.2
A = 1.0 / (np.sqrt(2.0) * BW)  # table arg scale: DerErf(A*(l-g)) = c*exp(-...)

GX = 96                       # x grid points (STT free dim — DVE cost ~ GX)
GY = 128                      # y grid points (matmul contraction dim)
C = GY // 128                 # gy partition chunks
ACT_TILES = 3                 # tiles whose U is ACT-copied PSUM->SBUF, so the
                              # DVE reduce skips the PSUM access init there

_prog_cache = {}
_jit_cache = {}


def _split_excess_waits(nc):
    """This walrus build rejects >1 sync wait per instruction ("Too many sync
    wait commands"). Hoist extra waits onto NoOps inserted immediately before
    the offending instruction on the same engine queue — the engine executes
    them in order, so the wait set is identical."""
    from concourse import mybir

    for f in nc.m.functions:
        for bb in f.blocks:
            out = []
            changed = False
            for inst in bb.instructions:
                si = inst.sync_info
                waits = list(si.on_wait) if si is not None else []
                if len(waits) > 1:
                    changed = True
                    for w in waits[:-1]:
                        nop = mybir.InstNoOp(
                            name=nc.get_next_instruction_name(),
                            sync_info=mybir.SyncInfo(on_wait=[w], on_update=[]),
                            bass_nofuse=True,
                            engine=inst.engine,
                        )
                        nc.register_instruction(nop)
                        out.append(nop)
                    si.on_wait = waits[-1:]
                    inst.sync_info = si
                out.append(inst)
            if changed:
                bb.instructions = out


def build_program(reps: int = 1):
    """One NeuronCore's program. Inputs (per core):
      xg8     [128, NT*GX]   f32: A*x_g tiled NT times (partition-identical)
      lxr     [128, NT*GX]   f32: A*lx_{t*128+p} repeated GX times per t
      ly_rep  [128, M_LOC]   f32: A*ly_m, every partition identical
      neg_yg  [128, C]       f32: -A*y_{c*128+p}
      wt      [128, B*C*GX]  bf16: col (b*C+c)*GX+gx -> W_b[gx, c*128+p]
    Output: out [128, B*N_TILES] f32, col t*B+b = sum_g Kx*Ky*W (unnormalized)
    """
    key = (reps, GX, GY, ACT_TILES)
    if key in _prog_cache:
        return _prog_cache[key]

    import concourse.bass as bass
    import concourse.tile as tile
    from concourse import mybir

    f32 = mybir.dt.float32
    bf16 = mybir.dt.bfloat16
    DerErf = mybir.ActivationFunctionType.Derivative_Erf

    nc = bass.Bass()
    xg_d = nc.dram_tensor("xg8", [128, N_TILES * GX], f32, kind="ExternalInput")
    lxr_d = nc.dram_tensor("lxr", [128, N_TILES * GX], f32, kind="ExternalInput")
    ly_d = nc.dram_tensor("ly_rep", [128, M_LOC], f32, kind="ExternalInput")
    nyg_d = nc.dram_tensor("neg_yg", [128, C], f32, kind="ExternalInput")
    wt_d = nc.dram_tensor("wt", [128, B * C * GX], bf16, kind="ExternalInput")
    out_d = nc.dram_tensor("out", [128, B * N_TILES], f32, kind="ExternalOutput")

    with tile.TileContext(nc) as tc:
        with (
            tc.tile_pool(name="consts", bufs=1) as consts,
            tc.tile_pool(name="tables", bufs=2) as tabp,
            tc.tile_pool(name="scratch", bufs=8) as scrp,
            tc.tile_pool(name="outp", bufs=2) as outp,
            tc.tile_pool(name="psum", bufs=8, space="PSUM") as psump,
        ):
            xg_t = consts.tile([128, N_TILES * GX], f32)
            lxr_t = consts.tile([128, N_TILES * GX], f32)
            ly_t = consts.tile([128, M_LOC], f32)
            nyg_t = consts.tile([128, C], f32)
            wt_t = consts.tile([128, B * C * GX], bf16)
            nc.sync.dma_start(xg_t[:], xg_d[:])
            nc.sync.dma_start(lxr_t[:], lxr_d[:])
            nc.sync.dma_start(ly_t[:], ly_d[:])
            nc.sync.dma_start(nyg_t[:], nyg_d[:])
            nc.sync.dma_start(wt_t[:], wt_d[:])

            for _ in range(reps):
                # 1-D Gaussian tables.  Ky: one ACT DerErf per gy chunk
                # (bias = per-partition -A*y_g).  Kx: the idle Pool engine
                # forms all 8 tiles' args (A*x_g - A*lx_m) in one tensor_sub,
                # then ONE big ACT DerErf converts them — 2 ACT ops total
                # instead of 9, cutting ACT busy from ~3.4us to ~2.1us.
                kyt_t = tabp.tile([128, C, M_LOC], bf16)  # [gy_p, c, m]
                args_t = tabp.tile([128, N_TILES, GX], f32)
                kx_t = tabp.tile([128, N_TILES, GX], f32)  # [m_p, t, gx]
                for c in range(C):
                    nc.scalar.activation(
                        out=kyt_t[:, c, :],
                        in_=ly_t[:],
                        func=DerErf,
                        bias=nyg_t[:, c : c + 1],
                        scale=1.0,
                    )
                nc.gpsimd.tensor_sub(args_t[:, :, :], xg_t[:], lxr_t[:])
                nc.scalar.activation(
                    out=kx_t[:, :, :],
                    in_=args_t[:, :, :],
                    func=DerErf,
                    scale=1.0,
                )

                out_sb = outp.tile([128, B * N_TILES], f32)
                for t in range(N_TILES):
                    # one matmul yields U for both batches: free dim = (b, gx)
                    ps = psump.tile([128, B, GX], f32)  # U[m_p, b, gx]
                    for c in range(C):
                        nc.tensor.matmul(
                            ps[:, :, :],
                            kyt_t[:, c, t * 128 : (t + 1) * 128],
                            wt_t[:],
                            start=(c == 0),
                            stop=(c == C - 1),
                        )
                    if t < ACT_TILES:
                        # ACT has slack (~1.5us) under the DVE bound: spend it
                        # staging this tile's U into SBUF so both STT inputs
                        # are SBUF (init 2x58 cycles instead of 2x120).
                        u_sb = scrp.tile([128, B, GX], f32)
                        nc.scalar.copy(u_sb[:, :, :], ps[:, :, :])
                        u_of = u_sb
                    else:
                        u_of = ps
                    for b in range(B):
                        prod = scrp.tile([128, GX], f32)
                        nc.vector.scalar_tensor_tensor(
                            out=prod[:],
                            in0=kx_t[:, t, :],
                            scalar=1.0,
                            in1=u_of[:, b, :],
                            op0=mybir.AluOpType.mult,
                            op1=mybir.AluOpType.mult,
                            accum_out=out_sb[:, t * B + b : t * B + b + 1],
                        )
                nc.sync.dma_start(out_d[:], out_sb[:])

    _split_excess_waits(nc)
    _prog_cache[key] = nc
    return nc


def make_in_maps(samples: np.ndarray, locations: np.ndarray):
    samples = np.asarray(samples, dtype=np.float32)
    locations = np.asarray(locations, dtype=np.float32)

    flat = samples.reshape(B * S, N)
    lo = flat.min(axis=0) - 1e-4
    hi = flat.max(axis=0) + 1e-4
    step_x = (hi[0] - lo[0]) / (GX - 1)
    step_y = (hi[1] - lo[1]) / (GY - 1)
    xg = (lo[0] + step_x * np.arange(GX)).astype(np.float64)
    yg = (lo[1] + step_y * np.arange(GY)).astype(np.float64)

    # linear binning per batch -> W_b[gx, gy]
    import ml_dtypes

    wt = np.empty((128, B * C * GX), dtype=ml_dtypes.bfloat16)
    for b in range(B):
        s = samples[b].astype(np.float64)
        fx = (s[:, 0] - lo[0]) / step_x
        fy = (s[:, 1] - lo[1]) / step_y
        ix = np.clip(np.floor(fx).astype(np.int64), 0, GX - 2)
        iy = np.clip(np.floor(fy).astype(np.int64), 0, GY - 2)
        wx = np.clip(fx - ix, 0.0, 1.0)
        wy = np.clip(fy - iy, 0.0, 1.0)
        W = np.zeros((GX, GY), dtype=np.float64)
        for dx, vx in ((0, 1.0 - wx), (1, wx)):
            for dy, vy in ((0, 1.0 - wy), (1, wy)):
                np.add.at(W, (ix + dx, iy + dy), vx * vy)
        # wt[p, (b*C+c)*GX+gx] = W[gx, c*128+p]
        wtb = W.T.astype(np.float32).reshape(C, 128, GX)  # [c, p, gx]
        for c in range(C):
            wt[:, (b * C + c) * GX : (b * C + c + 1) * GX] = wtb[c].astype(
                ml_dtypes.bfloat16
            )

    xg8 = np.broadcast_to(
        np.tile((A * xg).astype(np.float32), N_TILES), (128, N_TILES * GX)
    ).copy()
    neg_yg = (-A * yg).astype(np.float32).reshape(C, 128).T.copy()

    in_maps = []
    for core in range(N_CORES):
        lc = locations[core * M_LOC : (core + 1) * M_LOC]
        ly_rep = np.broadcast_to(
            (A * lc[:, 1]).astype(np.float32), (128, M_LOC)
        ).copy()
        lxt = (A * lc[:, 0]).astype(np.float32).reshape(N_TILES, 128).T  # [p, t]
        lxr = np.repeat(lxt[:, :, None], GX, axis=2).reshape(128, N_TILES * GX)
        in_maps.append(
            {
                "xg8": xg8,
                "lxr": np.ascontiguousarray(lxr),
                "ly_rep": ly_rep,
                "neg_yg": neg_yg,
                "wt": wt,
            }
        )
    return in_maps


def _get_runner(nc, n_cores):
    """Build the jitted shard_map once per program. run_bass_kernel_spmd
    re-lowers the whole BIR module on every call (fresh jit closure), which
    costs ~60us/instruction of host time per call; caching the jitted
    callable leaves only device execution + fixed dispatch."""
    key = (id(nc), n_cores)
    if key in _jit_cache:
        return _jit_cache[key]

    import jax
    from jax.sharding import Mesh, PartitionSpec
    from jax.experimental.shard_map import shard_map
    from concourse import mybir
    from concourse.bass2jax import (
        _bass_exec_p,
        partition_id_tensor,
        install_neuronx_cc_hook,
    )

    install_neuronx_cc_hook()
    partition_name = nc.partition_id_tensor.name if nc.partition_id_tensor else None
    in_names, out_names, out_avals, zero_shapes = [], [], [], []
    for alloc in nc.m.functions[0].allocations:
        if not isinstance(alloc, mybir.MemoryLocationSet):
            continue
        name = alloc.memorylocations[0].name
        if alloc.kind == "ExternalInput":
            if name != partition_name:
                in_names.append(name)
        elif alloc.kind == "ExternalOutput":
            shape = tuple(alloc.tensor_shape)
            dtype = mybir.dt.np(alloc.dtype)
            out_names.append(name)
            out_avals.append(jax.core.ShapedArray(shape, dtype))
            zero_shapes.append((shape, dtype))
    n_params = len(in_names)
    all_in_names = tuple(
        in_names + out_names + ([partition_name] if partition_name else [])
    )

    def _body(*args):
        operands = list(args)
        if partition_name is not None:
            operands.append(partition_id_tensor())
        outs = _bass_exec_p.bind(
            *operands,
            out_avals=tuple(out_avals),
            in_names=all_in_names,
            out_names=tuple(out_names),
            lowering_input_output_aliases=(),
            sim_require_finite=True,
            sim_require_nnan=True,
            nc=nc,
        )
        return tuple(outs)

    donate = tuple(range(n_params, n_params + len(out_names)))
    devices = jax.devices()[:n_cores]
    mesh = Mesh(np.asarray(devices), ("core",))
    in_specs = (PartitionSpec("core"),) * (n_params + len(out_names))
    out_specs = (PartitionSpec("core"),) * len(out_names)
    sharded = jax.jit(
        shard_map(
            _body, mesh=mesh, in_specs=in_specs, out_specs=out_specs,
            check_rep=False,
        ),
        donate_argnums=donate,
        keep_unused=True,
    )
    r = (sharded, in_names, out_names, out_avals, zero_shapes)
    _jit_cache[key] = r
    return r


_input_cache = {}


def run_on_cores(in_maps, reps: int = 1):
    import jax
    from jax.sharding import Mesh, NamedSharding, PartitionSpec

    nc = build_program(reps)
    sharded, in_names, out_names, out_avals, zero_shapes = _get_runner(nc, N_CORES)
    # Upload inputs once per (program, in_maps) pair — repeated timing calls
    # would otherwise re-ship the inputs over the axon tunnel every call.
    ikey = (id(nc), tuple(id(m[name]) for m in in_maps for name in in_names))
    concat_in = _input_cache.get(ikey)
    if concat_in is None:
        concat_in = [
            np.concatenate([np.asarray(m[name]) for m in in_maps], axis=0)
            for name in in_names
        ]
        try:
            devices = jax.devices()[:N_CORES]
            mesh = Mesh(np.asarray(devices), ("core",))
            sh = NamedSharding(mesh, PartitionSpec("core"))
            concat_in = [jax.device_put(a, sh) for a in concat_in]
        except Exception:
            pass
        _input_cache[ikey] = concat_in
    concat_zeros = [
        np.zeros((N_CORES * shape[0], *shape[1:]), dtype)
        for shape, dtype in zero_shapes
    ]
    out_arrs = sharded(*concat_in, *concat_zeros)
    return [
        {
            name: np.asarray(out_arrs[i]).reshape(N_CORES, *out_avals[i].shape)[c]
            for i, name in enumerate(out_names)
        }
        for c in range(N_CORES)
    ]


def kernel(samples: np.ndarray, locations: np.ndarray) -> np.ndarray:
    in_maps = make_in_maps(samples, locations)
    res = run_on_cores(in_maps, reps=1)
    # out core c: [128, B*N_TILES], col t*B+b -> m = c*M_LOC + t*128 + p
    out_full = np.empty((M, B), dtype=np.float32)
    for c in range(N_CORES):
        o = res[c]["out"]  # [128, 16]
        o = o.reshape(128, N_TILES, B).transpose(1, 0, 2)  # [t, p, b]
        out_full[c * M_LOC : (c + 1) * M_LOC] = o.reshape(M_LOC, B)
    norm = out_full.sum(axis=0)
    pdf = (out_full / norm.reshape(1, B)).reshape(1, M, B)
    return pdf.astype(np.float32)
